# revision 11
# baseline (speedup 1.0000x reference)
"""CopyGenerator kernel for 8 Trainium2 NeuronCores.

Sharding (collective-free):
  - Data-parallel over rows for the vocab projection: each core owns
    256 of the 2048 (tlen*bsz) rows and computes the FULL 32k-vocab
    projection + log_softmax locally.  The full W_out streams through
    SBUF in fp8-e4m3 (33 MB, overlapped with the matmuls), so no
    cross-core AllReduce (and no NEFF startup barrier) is needed --
    each core's on-silicon span is its own compute only.
  - Data-parallel over batch for the ext-vocab scatter: 4 of the 32
    batches per core, computed as a onehot matmul (iota + is_equal).

The projection runs fp8 x fp8 DoubleRow on the PE (fp32 PSUM, 2x
rate); the copy-gate z uses a separate fp16 matmul for precision.
The first half of the vocab keeps fp16 logits (finalized with a DVE
add of -ln(S)+ln(copy)); the second half keeps fp16 exp(logit) and is
finalized as Ln(scale * exp(logit)) on the scalar engine with a
per-row scale = clip(sigmoid(z)) / S, splitting the finalize tail
across both engines.  Host-side work is layout marshalling only.
"""
import sys
sys.path.insert(0, "/opt/trn_rl_repo")
import numpy as np

TLEN, BSZ, HID = 64, 32, 1024
SLEN, V_TGT, V_EXT = 200, 32000, 2000
NCORES = 8
NROWS = TLEN * BSZ             # 2048
RSH = NROWS // NCORES          # 256 rows per core
NTL = RSH // 128               # 2 row tiles per core
BSH = BSZ // NCORES            # 4 batches per core (ext scatter)
KB = HID // 128                # 8 contraction chunks (4 DoubleRow pairs)
VPAD = 512                     # vocab chunk width (padded)
NVC = 63                       # 62 full chunks + 1 chunk of 256
VLAST = V_TGT - 62 * VPAD      # 256 valid cols in the last chunk
NVP = (NVC + 1) // 2           # 32 chunk pairs (last pair is a single)
LGP = 16                       # first 16 pairs keep logits (DVE finalize)
LGW = LGP * 2 * VPAD           # 16384 cols in the logit half
ESW = V_TGT - LGW              # 15616 cols in the exp half
FC = 2048                      # finalize chunk width
FSL_LG = [slice(i * FC, (i + 1) * FC) for i in range(LGW // FC)]
FSL_ES = [slice(i * FC, min(ESW, (i + 1) * FC))
          for i in range(-(-ESW // FC))]
SA, SB_ = 128, SLEN - 128      # source-len split (128 + 72)
EC = 500                       # ext chunk
NEC = V_EXT // EC              # 4
LOG_LO = float(np.log(0.001))

_prog_cache = {}


def _build_program(has_bout: bool, neg_bcopy: float):
    import concourse.bacc as bacc
    import concourse.tile as tile
    import concourse.mybir as mybir

    f32, f16, i32 = mybir.dt.float32, mybir.dt.float16, mybir.dt.int32
    f8 = mybir.dt.float8e4
    AF = mybir.ActivationFunctionType
    OP = mybir.AluOpType
    PM = mybir.MatmulPerfMode

    nc = bacc.Bacc("TRN2", target_bir_lowering=False, debug=False,
                   num_devices=NCORES)

    # tile-contiguous host layouts
    WTh = nc.dram_tensor("WTh", [NVC, 128, KB, VPAD], f8, kind="ExternalInput")
    hT8 = nc.dram_tensor("hT8", [NTL, 128, KB, 128], f8, kind="ExternalInput")
    hTh = nc.dram_tensor("hTh", [NTL, 128, KB, 128], f16, kind="ExternalInput")
    wcT = nc.dram_tensor("wcT", [128, KB], f16, kind="ExternalInput")
    attnT = nc.dram_tensor("attnT", [BSH, SLEN, TLEN], f16, kind="ExternalInput")
    idxc = nc.dram_tensor("idxc", [BSH, SLEN], i32, kind="ExternalInput")
    hxT = nc.dram_tensor("hxT", [BSH, 128, KB, TLEN], f16, kind="ExternalInput")
    if has_bout:
        bbh = nc.dram_tensor("bbh", [1, NVC, VPAD], f16, kind="ExternalInput")
    vout = nc.dram_tensor("vout", [NTL, 128, V_TGT], f16, kind="ExternalOutput")
    eout = nc.dram_tensor("eout", [TLEN, BSH, V_EXT], f32, kind="ExternalOutput")

    with tile.TileContext(nc) as tc:
        with (
            tc.tile_pool(name="lg", bufs=NTL) as lg_pool,
            tc.tile_pool(name="es", bufs=NTL) as es_pool,
            tc.tile_pool(name="wt", bufs=3) as wt_pool,
            tc.tile_pool(name="ht", bufs=2 * NTL) as ht_pool,
            tc.tile_pool(name="const", bufs=1) as const_pool,
            tc.tile_pool(name="esc", bufs=2) as esc_pool,
            tc.tile_pool(name="sep", bufs=NTL) as sep_pool,
            tc.tile_pool(name="stage", bufs=4) as stage_pool,
            tc.tile_pool(name="small", bufs=8) as small_pool,
            tc.tile_pool(name="ext", bufs=2) as ext_pool,
            tc.tile_pool(name="psA", bufs=3, space="PSUM") as psA_pool,
            tc.tile_pool(name="psB", bufs=2, space="PSUM") as psB_pool,
        ):
            # ---- prologue loads --------------------------------------
            ht8, ht = {}, {}
            for t in range(NTL):
                ht8[t] = ht_pool.tile([128, KB, 128], f8, tag="ht8",
                                      name=f"ht8_{t}")
                nc.gpsimd.dma_start(ht8[t][:], hT8[t])
                ht[t] = ht_pool.tile([128, KB, 128], f16, tag="ht",
                                     name=f"ht{t}")
                nc.gpsimd.dma_start(ht[t][:], hTh[t])
            wc_sb = const_pool.tile([128, KB], f16)
            nc.gpsimd.dma_start(wc_sb[:], wcT[:])
            if has_bout:
                bb_sb = const_pool.tile([1, NVC, VPAD], f16)
                nc.gpsimd.dma_start(bb_sb[:], bbh[0])
                ones1 = const_pool.tile([1, 128], f16)
                nc.vector.memset(ones1[:], 1.0)

            # logits (first half) / exp-logits (second half) per tile
            lg = {t: lg_pool.tile([128, LGW], f16, tag="lg", name=f"lg{t}")
                  for t in range(NTL)}
            es = {t: es_pool.tile([128, ESW], f16, tag="es", name=f"es{t}")
                  for t in range(NTL)}
            sep = {t: sep_pool.tile([128, NVP], f32, tag="sep",
                                    name=f"sep{t}") for t in range(NTL)}

            # copy-gate z via fp16 matmul (separate from fp8 stream)
            zc = {}
            for t in range(NTL):
                zp = psB_pool.tile([128, VPAD], f32, tag="pm", name=f"zp{t}")
                for kb in range(KB):
                    nc.tensor.matmul(zp[:, :1], ht[t][:, kb, :],
                                     wc_sb[:, kb:kb + 1],
                                     start=(kb == 0), stop=(kb == KB - 1))
                e_t = small_pool.tile([128, 1], f32, tag="e", name=f"e{t}")
                nc.scalar.activation(e_t[:], zp[:, :1], AF.Exp,
                                     scale=-1.0, bias=neg_bcopy)
                zc[t] = e_t

            # ---- main loop: stream W chunk pairs, both row tiles -----
            for vp in range(NVP):
                vcs = [2 * vp] + ([2 * vp + 1] if 2 * vp + 1 < NVC else [])
                ws = []
                for vc in vcs:
                    w = wt_pool.tile([128, KB, VPAD], f8, tag="wt",
                                     name=f"wt{vc}")
                    nc.gpsimd.dma_start(w[:], WTh[vc])
                    ws.append(w)
                wtot = sum(VLAST if vc == NVC - 1 else VPAD for vc in vcs)
                for t in range(NTL):
                    pm = psA_pool.tile([128, 2 * VPAD], f32, tag="pm2",
                                       name=f"pm{vp}_{t}")
                    for kp in range(KB // 2):
                        for i, (vc, w) in enumerate(zip(vcs, ws)):
                            nc.tensor.matmul(
                                pm[:, i * VPAD:(i + 1) * VPAD],
                                ht8[t][:, 2 * kp:2 * kp + 2, :],
                                w[:, 2 * kp:2 * kp + 2, :],
                                start=(kp == 0),
                                stop=(kp == KB // 2 - 1 and not has_bout),
                                perf_mode=PM.DoubleRow)
                    if has_bout:
                        for i, vc in enumerate(vcs):
                            nc.tensor.matmul(pm[:, i * VPAD:(i + 1) * VPAD],
                                             ones1[:], bb_sb[:, vc],
                                             start=False, stop=True)
                    if vp < LGP:
                        # logit half: fp16 logits (DVE) + exp row-sums (ACT)
                        base = vp * 2 * VPAD
                        nc.vector.tensor_copy(lg[t][:, base:base + wtot],
                                              pm[:, :wtot])
                        esc = esc_pool.tile([128, 2 * VPAD], f16, tag="esc",
                                            name=f"esc{vp}_{t}", bufs=1)
                        nc.scalar.activation(esc[:, :wtot], pm[:, :wtot],
                                             AF.Exp,
                                             accum_out=sep[t][:, vp:vp + 1])
                    else:
                        # exp half: persist exp(logit) directly (ACT only)
                        base = (vp - LGP) * 2 * VPAD
                        nc.scalar.activation(es[t][:, base:base + wtot],
                                             pm[:, :wtot], AF.Exp,
                                             accum_out=sep[t][:, vp:vp + 1])

            # ---- gate + normalizer per row tile ----------------------
            scl, negc = {}, {}
            for t in range(NTL):
                sp = small_pool.tile([128, 1], f32, tag="sp", name=f"sp{t}")
                nc.vector.tensor_scalar_add(sp[:], zc[t][:], 1.0)
                sig = small_pool.tile([128, 1], f32, tag="sig", name=f"sig{t}")
                nc.vector.reciprocal(sig[:], sp[:])
                cl = small_pool.tile([128, 1], f32, tag="cl", name=f"cl{t}")
                nc.vector.tensor_scalar(cl[:], sig[:], 0.001, 0.999,
                                        op0=OP.max, op1=OP.min)
                lcs = small_pool.tile([128, 1], f32, tag="lcs", name=f"lcs{t}")
                nc.scalar.activation(lcs[:], cl[:], AF.Ln)
                ssum = small_pool.tile([128, 1], f32, tag="ssum",
                                       name=f"ssum{t}")
                nc.vector.tensor_reduce(ssum[:], sep[t][:],
                                        axis=mybir.AxisListType.X, op=OP.add)
                lns = small_pool.tile([128, 1], f32, tag="lns", name=f"lns{t}")
                nc.scalar.activation(lns[:], ssum[:], AF.Ln)
                ng = small_pool.tile([128, 1], f32, tag="negc", name=f"negc{t}")
                nc.vector.tensor_sub(ng[:], lcs[:], lns[:])
                negc[t] = ng
                invs = small_pool.tile([128, 1], f32, tag="invs",
                                       name=f"invs{t}")
                nc.vector.reciprocal(invs[:], ssum[:])
                sc = small_pool.tile([128, 1], f32, tag="scl", name=f"scl{t}")
                nc.vector.tensor_mul(sc[:], cl[:], invs[:])
                scl[t] = sc

            # ---- ext-vocab scatter (batch-sharded) --------------------
            iota_sb = const_pool.tile([128, V_EXT], f32)
            nc.gpsimd.iota(iota_sb[:], pattern=[[1, V_EXT]], base=0,
                           channel_multiplier=0,
                           allow_small_or_imprecise_dtypes=True)
            for b in range(BSH):
                hx_sb = ext_pool.tile([128, KB, TLEN], f16, tag="hx")
                nc.gpsimd.dma_start(hx_sb[:], hxT[b])
                zx = psB_pool.tile([128, VPAD], f32, tag="pm", name=f"zx{b}")
                for kb in range(KB):
                    nc.tensor.matmul(zx[:TLEN, :1], hx_sb[:, kb, :],
                                     wc_sb[:, kb:kb + 1],
                                     start=(kb == 0), stop=(kb == KB - 1))
                ex = small_pool.tile([TLEN, 1], f32, tag="ex", name=f"ex{b}")
                nc.scalar.activation(ex[:], zx[:TLEN, :1], AF.Exp,
                                     scale=-1.0, bias=neg_bcopy)
                spx = small_pool.tile([TLEN, 1], f32, tag="spx", name=f"spx{b}")
                nc.vector.tensor_scalar_add(spx[:], ex[:], 1.0)
                ivx = small_pool.tile([TLEN, 1], f32, tag="ivx", name=f"ivx{b}")
                nc.vector.reciprocal(ivx[:], spx[:])
                sgx = small_pool.tile([TLEN, 1], f32, tag="sgx", name=f"sgx{b}")
                nc.vector.tensor_mul(sgx[:], ex[:], ivx[:])   # 1 - sigmoid

                idx_i = ext_pool.tile([128, 2], i32, tag="idxi")
                nc.sync.dma_start(idx_i[:SA, 0:1],
                                  idxc[b:b + 1, 0:SA].rearrange("o s -> s o"))
                nc.sync.dma_start(idx_i[:SB_, 1:2],
                                  idxc[b:b + 1, SA:SLEN].rearrange("o s -> s o"))
                idx_sb = ext_pool.tile([128, 2], f32, tag="idx")
                nc.vector.tensor_copy(idx_sb[:SA, 0:1], idx_i[:SA, 0:1])
                nc.vector.tensor_copy(idx_sb[:SB_, 1:2], idx_i[:SB_, 1:2])
                oh_a = ext_pool.tile([128, V_EXT], f16, tag="oha", bufs=1)
                oh_b = ext_pool.tile([128, V_EXT], f16, tag="ohb", bufs=1)
                nc.vector.tensor_scalar(oh_a[:], iota_sb[:], idx_sb[:, 0:1],
                                        None, op0=OP.is_equal)
                nc.vector.tensor_scalar(oh_b[:SB_], iota_sb[:SB_],
                                        idx_sb[:SB_, 1:2], None,
                                        op0=OP.is_equal)

                at_a = ext_pool.tile([128, TLEN], f16, tag="ata")
                at_b = ext_pool.tile([128, TLEN], f16, tag="atb")
                nc.gpsimd.dma_start(at_a[:], attnT[b, 0:SA, :])
                nc.gpsimd.dma_start(at_b[:SB_], attnT[b, SA:SLEN, :])

                for ec in range(NEC):
                    sl = slice(ec * EC, (ec + 1) * EC)
                    pe_ = psB_pool.tile([128, VPAD], f32, tag="pm",
                                        name=f"pe{b}_{ec}")
                    nc.tensor.matmul(pe_[:TLEN, :EC], at_a[:], oh_a[:, sl],
                                     start=True, stop=False)
                    nc.tensor.matmul(pe_[:TLEN, :EC], at_b[:SB_], oh_b[:SB_, sl],
                                     start=False, stop=True)
                    est = stage_pool.tile([TLEN, EC], f32, tag="est",
                                          name=f"est{b}_{ec}", bufs=2)
                    nc.vector.tensor_scalar(est[:], pe_[:TLEN, :EC], sgx[:],
                                            0.001, op0=OP.mult, op1=OP.max)
                    nc.vector.tensor_scalar_min(est[:], est[:], 0.999)
                    elg = stage_pool.tile([TLEN, EC], f32, tag="elg",
                                          name=f"elg{b}_{ec}", bufs=2)
                    nc.scalar.activation(elg[:], est[:], AF.Ln)
                    if ec == 0:
                        nc.vector.memset(elg[:, 0:1], LOG_LO)
                    nc.sync.dma_start(eout[:, b, sl], elg[:])

            # ---- finalize: DVE add on logit half, ACT Ln on exp half -
            for t in range(NTL):
                for fc, sl in enumerate(FSL_LG):
                    st = stage_pool.tile([128, FC], f16, tag="st",
                                         name=f"st{t}_{fc}", bufs=2)
                    nc.vector.tensor_scalar_add(st[:], lg[t][:, sl], negc[t][:])
                    nc.sync.dma_start(vout[t, :, sl], st[:])
                for fc, sl in enumerate(FSL_ES):
                    w_ = sl.stop - sl.start
                    st = stage_pool.tile([128, FC], f16, tag="su",
                                         name=f"su{t}_{fc}", bufs=2)
                    nc.scalar.activation(st[:, :w_], es[t][:, sl], AF.Ln,
                                         scale=scl[t][:])
                    nc.sync.dma_start(vout[t, :, LGW + sl.start:LGW + sl.stop],
                                      st[:, :w_])

    nc.compile()
    return nc


def _get_program(has_bout: bool, neg_bcopy: float):
    key = (has_bout, neg_bcopy)
    if key not in _prog_cache:
        _prog_cache[key] = _build_program(has_bout, neg_bcopy)
    return _prog_cache[key]


def _marshal(hidden, attn, copy_to_ext, W_out, b_out, w_copy, b_copy):
    import ml_dtypes
    f8 = ml_dtypes.float8_e4m3

    h2 = np.asarray(hidden, np.float32).reshape(NROWS, HID)
    a2 = np.asarray(attn, np.float32)
    attnT_full = np.ascontiguousarray(
        a2.transpose(1, 2, 0)).astype(np.float16)              # [32, 200, 64]
    idx_full = np.ascontiguousarray(
        np.asarray(copy_to_ext).astype(np.int32).T)            # [32, 200]
    W8 = np.asarray(W_out, np.float32).astype(f8)              # [32000, 1024]
    wc16 = np.asarray(w_copy, np.float32).astype(
        np.float16).reshape(HID)
    bo = np.asarray(b_out, np.float32)
    neg_bcopy = -float(np.asarray(b_copy, np.float32).reshape(-1)[0])
    has_bout = bool(np.any(bo))

    # shared W^T chunks: WTh[vc, p, kb, j] = W.T[kb*128+p, vc*512+j]
    Wt = W8.T                                                  # [1024, 32000]
    full = np.zeros((HID, NVC, VPAD), f8)
    full[:, :NVC - 1, :] = Wt[:, :62 * VPAD].reshape(HID, 62, VPAD)
    full[:, NVC - 1, :VLAST] = Wt[:, 62 * VPAD:]
    WTh = np.ascontiguousarray(
        full.reshape(KB, 128, NVC, VPAD).transpose(2, 1, 0, 3))
    wcT = np.ascontiguousarray(wc16.reshape(KB, 128).T)        # [128, KB]
    if has_bout:
        bbh = np.zeros((1, NVC, VPAD), np.float16)
        bbh[0, :NVC - 1, :] = bo[:62 * VPAD].reshape(62, VPAD)
        bbh[0, NVC - 1, :VLAST] = bo[62 * VPAD:]

    in_maps = []
    for c in range(NCORES):
        # hTh[t, p, kb, r] = h2[c*256 + t*128 + r, kb*128 + p]
        hc = h2[c * RSH:(c + 1) * RSH]
        hcT = np.ascontiguousarray(
            hc.reshape(NTL, 128, KB, 128).transpose(0, 3, 2, 1))
        # hxT[b, p, kb, t] = h2[t*BSZ + (c*BSH+b), kb*128+p]
        hx = np.stack([np.ascontiguousarray(
            h2[(c * BSH + b)::BSZ, :].astype(np.float16)
            .reshape(TLEN, KB, 128).transpose(2, 1, 0)) for b in range(BSH)])
        bsl = slice(c * BSH, (c + 1) * BSH)
        m = {
            "WTh": WTh,
            "hT8": hcT.astype(f8),
            "hTh": hcT.astype(np.float16),
            "wcT": wcT,
            "attnT": np.ascontiguousarray(attnT_full[bsl]),
            "idxc": np.ascontiguousarray(idx_full[bsl]),
            "hxT": hx,
        }
        if has_bout:
            m["bbh"] = bbh
        in_maps.append(m)
    return in_maps, has_bout, neg_bcopy


def _assemble(results):
    out = np.empty((NROWS, V_TGT + V_EXT), np.float32)
    out3 = out.reshape(TLEN, BSZ, V_TGT + V_EXT)
    for c in range(NCORES):
        out[c * RSH:(c + 1) * RSH, :V_TGT] = \
            results[c]["vout"].reshape(RSH, V_TGT)
        out3[:, c * BSH:(c + 1) * BSH, V_TGT:] = results[c]["eout"]
    return out3


LAST_RES = None


def kernel(hidden, attn, copy_to_ext, W_out, b_out, w_copy, b_copy):
    global LAST_RES
    from concourse.bass_utils import run_bass_kernel_spmd

    in_maps, has_bout, neg_bcopy = _marshal(
        hidden, attn, copy_to_ext, W_out, b_out, w_copy, b_copy)
    nc = _get_program(has_bout, neg_bcopy)
    res = run_bass_kernel_spmd(nc, in_maps, core_ids=list(range(NCORES)))
    LAST_RES = res
    return _assemble(res.results)


# revision 14
# speedup vs baseline: 1.3489x; 1.3489x over previous
"""CopyGenerator kernel for 8 Trainium2 NeuronCores.

Sharding (collective-free):
  - Data-parallel over rows for the vocab projection: each core owns
    256 of the 2048 (tlen*bsz) rows and computes the FULL 32k-vocab
    projection + log_softmax locally.  The full W_out streams through
    SBUF in fp8-e4m3 (33 MB, overlapped with the matmuls), so no
    cross-core AllReduce (and no NEFF startup barrier) is needed --
    each core's on-silicon span is its own compute only.
  - Data-parallel over batch for the ext-vocab scatter: 4 of the 32
    batches per core, computed as a onehot matmul (iota + is_equal).

The projection runs fp8 x fp8 DoubleRow on the PE (fp32 PSUM, 2x
rate); the copy-gate z uses a separate fp16 matmul for precision.
The first half of the vocab keeps fp16 logits (finalized with a DVE
add of -ln(S)+ln(copy)); the second half keeps fp16 exp(logit) and is
finalized as Ln(scale * exp(logit)) on the scalar engine with a
per-row scale = clip(sigmoid(z)) / S, splitting the finalize tail
across both engines.  Host-side work is layout marshalling only.
"""
import sys
sys.path.insert(0, "/opt/trn_rl_repo")
import numpy as np

TLEN, BSZ, HID = 64, 32, 1024
SLEN, V_TGT, V_EXT = 200, 32000, 2000
NCORES = 8
NROWS = TLEN * BSZ             # 2048
RSH = NROWS // NCORES          # 256 rows per core
NTL = RSH // 128               # 2 row tiles per core
BSH = BSZ // NCORES            # 4 batches per core (ext scatter)
KB = HID // 128                # 8 contraction chunks (4 DoubleRow pairs)
VPAD = 512                     # vocab chunk width (padded)
NVC = 63                       # 62 full chunks + 1 chunk of 256
VLAST = V_TGT - 62 * VPAD      # 256 valid cols in the last chunk
NVP = (NVC + 1) // 2           # 32 chunk pairs (last pair is a single)
LGP = 16                       # first 16 pairs keep logits (DVE finalize)
LGW = LGP * 2 * VPAD           # 16384 cols in the logit half
ESW = V_TGT - LGW              # 15616 cols in the exp half
FC = 2048                      # finalize chunk width
FSL_LG = [slice(i * FC, (i + 1) * FC) for i in range(LGW // FC)]
FSL_ES = [slice(i * FC, min(ESW, (i + 1) * FC))
          for i in range(-(-ESW // FC))]
SA, SB_ = 128, SLEN - 128      # source-len split (128 + 72)
EC = 500                       # ext chunk
NEC = V_EXT // EC              # 4
LOG_LO = float(np.log(0.001))

_prog_cache = {}


def _build_program(has_bout: bool, neg_bcopy: float):
    import concourse.bacc as bacc
    import concourse.tile as tile
    import concourse.mybir as mybir

    f32, f16, i32 = mybir.dt.float32, mybir.dt.float16, mybir.dt.int32
    f8 = mybir.dt.float8e4
    AF = mybir.ActivationFunctionType
    OP = mybir.AluOpType
    PM = mybir.MatmulPerfMode

    nc = bacc.Bacc("TRN2", target_bir_lowering=False, debug=False,
                   num_devices=NCORES)

    # tile-contiguous host layouts
    WTh = nc.dram_tensor("WTh", [NVC, 128, KB, VPAD], f8, kind="ExternalInput")
    hT8 = nc.dram_tensor("hT8", [NTL, 128, KB, 128], f8, kind="ExternalInput")
    hTh = nc.dram_tensor("hTh", [NTL, 128, KB, 128], f16, kind="ExternalInput")
    wcT = nc.dram_tensor("wcT", [128, KB], f16, kind="ExternalInput")
    attnT = nc.dram_tensor("attnT", [BSH, SLEN, TLEN], f16, kind="ExternalInput")
    idxc = nc.dram_tensor("idxc", [BSH, SLEN], i32, kind="ExternalInput")
    hxT = nc.dram_tensor("hxT", [BSH, 128, KB, TLEN], f16, kind="ExternalInput")
    if has_bout:
        bbh = nc.dram_tensor("bbh", [1, NVC, VPAD], f16, kind="ExternalInput")
    vout = nc.dram_tensor("vout", [NTL, 128, V_TGT], f16, kind="ExternalOutput")
    eout = nc.dram_tensor("eout", [TLEN, BSH, V_EXT], f32, kind="ExternalOutput")

    with tile.TileContext(nc) as tc:
        with (
            tc.tile_pool(name="lg", bufs=NTL) as lg_pool,
            tc.tile_pool(name="es", bufs=NTL) as es_pool,
            tc.tile_pool(name="wt", bufs=3) as wt_pool,
            tc.tile_pool(name="ht", bufs=2 * NTL) as ht_pool,
            tc.tile_pool(name="const", bufs=1) as const_pool,
            tc.tile_pool(name="esc", bufs=2) as esc_pool,
            tc.tile_pool(name="sep", bufs=NTL) as sep_pool,
            tc.tile_pool(name="stage", bufs=4) as stage_pool,
            tc.tile_pool(name="small", bufs=8) as small_pool,
            tc.tile_pool(name="ext", bufs=2) as ext_pool,
            tc.tile_pool(name="psA", bufs=3, space="PSUM") as psA_pool,
            tc.tile_pool(name="psB", bufs=2, space="PSUM") as psB_pool,
        ):
            # ---- prologue loads --------------------------------------
            ht8, ht = {}, {}
            for t in range(NTL):
                ht8[t] = ht_pool.tile([128, KB, 128], f8, tag="ht8",
                                      name=f"ht8_{t}")
                nc.gpsimd.dma_start(ht8[t][:], hT8[t])
                ht[t] = ht_pool.tile([128, KB, 128], f16, tag="ht",
                                     name=f"ht{t}")
                nc.gpsimd.dma_start(ht[t][:], hTh[t])
            wc_sb = const_pool.tile([128, KB], f16)
            nc.gpsimd.dma_start(wc_sb[:], wcT[:])
            if has_bout:
                bb_sb = const_pool.tile([1, NVC, VPAD], f16)
                nc.gpsimd.dma_start(bb_sb[:], bbh[0])
                ones1 = const_pool.tile([1, 128], f16)
                nc.vector.memset(ones1[:], 1.0)

            # logits (first half) / exp-logits (second half) per tile
            lg = {t: lg_pool.tile([128, LGW], f16, tag="lg", name=f"lg{t}")
                  for t in range(NTL)}
            es = {t: es_pool.tile([128, ESW], f16, tag="es", name=f"es{t}")
                  for t in range(NTL)}
            sep = {t: sep_pool.tile([128, NVP], f32, tag="sep",
                                    name=f"sep{t}") for t in range(NTL)}

            # copy-gate z via fp16 matmul (separate from fp8 stream)
            zc = {}
            for t in range(NTL):
                zp = psB_pool.tile([128, VPAD], f32, tag="pm", name=f"zp{t}")
                for kb in range(KB):
                    nc.tensor.matmul(zp[:, :1], ht[t][:, kb, :],
                                     wc_sb[:, kb:kb + 1],
                                     start=(kb == 0), stop=(kb == KB - 1))
                e_t = small_pool.tile([128, 1], f32, tag="e", name=f"e{t}")
                nc.scalar.activation(e_t[:], zp[:, :1], AF.Exp,
                                     scale=-1.0, bias=neg_bcopy)
                zc[t] = e_t

            # ext gate (1 - sigmoid) per batch, early so its Exp shares
            # the main Exp table epoch
            sgxs = {}
            for b in range(BSH):
                hx_sb = ext_pool.tile([128, KB, TLEN], f16, tag="hx")
                nc.gpsimd.dma_start(hx_sb[:], hxT[b])
                zx = psB_pool.tile([128, VPAD], f32, tag="pm", name=f"zx{b}")
                for kb in range(KB):
                    nc.tensor.matmul(zx[:TLEN, :1], hx_sb[:, kb, :],
                                     wc_sb[:, kb:kb + 1],
                                     start=(kb == 0), stop=(kb == KB - 1))
                ex = small_pool.tile([TLEN, 1], f32, tag="ex", name=f"ex{b}")
                nc.scalar.activation(ex[:], zx[:TLEN, :1], AF.Exp,
                                     scale=-1.0, bias=neg_bcopy)
                spx = small_pool.tile([TLEN, 1], f32, tag="spx", name=f"spx{b}")
                nc.vector.tensor_scalar_add(spx[:], ex[:], 1.0)
                ivx = small_pool.tile([TLEN, 1], f32, tag="ivx", name=f"ivx{b}")
                nc.vector.reciprocal(ivx[:], spx[:])
                sgx = small_pool.tile([TLEN, 1], f32, tag="sgx", name=f"sgx{b}")
                nc.vector.tensor_mul(sgx[:], ex[:], ivx[:])   # 1 - sigmoid
                sgxs[b] = sgx

            # ---- main loop: stream W chunk pairs, both row tiles -----
            for vp in range(NVP):
                vcs = [2 * vp] + ([2 * vp + 1] if 2 * vp + 1 < NVC else [])
                ws = []
                for vc in vcs:
                    w = wt_pool.tile([128, KB, VPAD], f8, tag="wt",
                                     name=f"wt{vc}")
                    nc.gpsimd.dma_start(w[:], WTh[vc])
                    ws.append(w)
                wtot = sum(VLAST if vc == NVC - 1 else VPAD for vc in vcs)
                for t in range(NTL):
                    pm = psA_pool.tile([128, 2 * VPAD], f32, tag="pm2",
                                       name=f"pm{vp}_{t}")
                    for i, (vc, w) in enumerate(zip(vcs, ws)):
                        for kp in range(KB // 2):
                            nc.tensor.matmul(
                                pm[:, i * VPAD:(i + 1) * VPAD],
                                ht8[t][:, 2 * kp:2 * kp + 2, :],
                                w[:, 2 * kp:2 * kp + 2, :],
                                start=(kp == 0),
                                stop=(kp == KB // 2 - 1 and not has_bout),
                                perf_mode=PM.DoubleRow)
                    if has_bout:
                        for i, vc in enumerate(vcs):
                            nc.tensor.matmul(pm[:, i * VPAD:(i + 1) * VPAD],
                                             ones1[:], bb_sb[:, vc],
                                             start=False, stop=True)
                    if vp < LGP:
                        # logit half: fp16 logits (DVE) + exp row-sums (ACT)
                        base = vp * 2 * VPAD
                        nc.vector.tensor_copy(lg[t][:, base:base + wtot],
                                              pm[:, :wtot])
                        esc = esc_pool.tile([128, 2 * VPAD], f16, tag="esc",
                                            name=f"esc{vp}_{t}", bufs=1)
                        nc.scalar.activation(esc[:, :wtot], pm[:, :wtot],
                                             AF.Exp,
                                             accum_out=sep[t][:, vp:vp + 1])
                    else:
                        # exp half: persist exp(logit) directly (ACT only)
                        base = (vp - LGP) * 2 * VPAD
                        nc.scalar.activation(es[t][:, base:base + wtot],
                                             pm[:, :wtot], AF.Exp,
                                             accum_out=sep[t][:, vp:vp + 1])

            # ---- gate + normalizer per row tile ----------------------
            scl, negc = {}, {}
            for t in range(NTL):
                sp = small_pool.tile([128, 1], f32, tag="sp", name=f"sp{t}")
                nc.vector.tensor_scalar_add(sp[:], zc[t][:], 1.0)
                sig = small_pool.tile([128, 1], f32, tag="sig", name=f"sig{t}")
                nc.vector.reciprocal(sig[:], sp[:])
                cl = small_pool.tile([128, 1], f32, tag="cl", name=f"cl{t}")
                nc.vector.tensor_scalar(cl[:], sig[:], 0.001, 0.999,
                                        op0=OP.max, op1=OP.min)
                lcs = small_pool.tile([128, 1], f32, tag="lcs", name=f"lcs{t}")
                nc.scalar.activation(lcs[:], cl[:], AF.Ln)
                ssum = small_pool.tile([128, 1], f32, tag="ssum",
                                       name=f"ssum{t}")
                nc.vector.tensor_reduce(ssum[:], sep[t][:],
                                        axis=mybir.AxisListType.X, op=OP.add)
                lns = small_pool.tile([128, 1], f32, tag="lns", name=f"lns{t}")
                nc.scalar.activation(lns[:], ssum[:], AF.Ln)
                ng = small_pool.tile([128, 1], f32, tag="negc", name=f"negc{t}")
                nc.vector.tensor_sub(ng[:], lcs[:], lns[:])
                negc[t] = ng
                invs = small_pool.tile([128, 1], f32, tag="invs",
                                       name=f"invs{t}")
                nc.vector.reciprocal(invs[:], ssum[:])
                sc = small_pool.tile([128, 1], f32, tag="scl", name=f"scl{t}")
                nc.vector.tensor_mul(sc[:], cl[:], invs[:])
                scl[t] = sc

            # ---- ext-vocab scatter (batch-sharded) --------------------
            iota_sb = const_pool.tile([128, V_EXT], f32)
            nc.gpsimd.iota(iota_sb[:], pattern=[[1, V_EXT]], base=0,
                           channel_multiplier=0,
                           allow_small_or_imprecise_dtypes=True)
            for b in range(BSH):
                sgx = sgxs[b]
                idx_i = ext_pool.tile([128, 2], i32, tag="idxi")
                nc.sync.dma_start(idx_i[:SA, 0:1],
                                  idxc[b:b + 1, 0:SA].rearrange("o s -> s o"))
                nc.sync.dma_start(idx_i[:SB_, 1:2],
                                  idxc[b:b + 1, SA:SLEN].rearrange("o s -> s o"))
                idx_sb = ext_pool.tile([128, 2], f32, tag="idx")
                nc.vector.tensor_copy(idx_sb[:SA, 0:1], idx_i[:SA, 0:1])
                nc.vector.tensor_copy(idx_sb[:SB_, 1:2], idx_i[:SB_, 1:2])
                oh_a = ext_pool.tile([128, V_EXT], f16, tag="oha", bufs=1)
                oh_b = ext_pool.tile([128, V_EXT], f16, tag="ohb", bufs=1)
                nc.vector.tensor_scalar(oh_a[:], iota_sb[:], idx_sb[:, 0:1],
                                        None, op0=OP.is_equal)
                nc.vector.tensor_scalar(oh_b[:SB_], iota_sb[:SB_],
                                        idx_sb[:SB_, 1:2], None,
                                        op0=OP.is_equal)

                at_a = ext_pool.tile([128, TLEN], f16, tag="ata")
                at_b = ext_pool.tile([128, TLEN], f16, tag="atb")
                nc.gpsimd.dma_start(at_a[:], attnT[b, 0:SA, :])
                nc.gpsimd.dma_start(at_b[:SB_], attnT[b, SA:SLEN, :])

                for ec in range(NEC):
                    sl = slice(ec * EC, (ec + 1) * EC)
                    pe_ = psB_pool.tile([128, VPAD], f32, tag="pm",
                                        name=f"pe{b}_{ec}")
                    nc.tensor.matmul(pe_[:TLEN, :EC], at_a[:], oh_a[:, sl],
                                     start=True, stop=False)
                    nc.tensor.matmul(pe_[:TLEN, :EC], at_b[:SB_], oh_b[:SB_, sl],
                                     start=False, stop=True)
                    est = stage_pool.tile([TLEN, EC], f32, tag="est",
                                          name=f"est{b}_{ec}", bufs=2)
                    nc.vector.tensor_scalar(est[:], pe_[:TLEN, :EC], sgx[:],
                                            0.001, op0=OP.mult, op1=OP.max)
                    nc.vector.tensor_scalar_min(est[:], est[:], 0.999)
                    elg = stage_pool.tile([TLEN, EC], f32, tag="elg",
                                          name=f"elg{b}_{ec}", bufs=2)
                    nc.scalar.activation(elg[:], est[:], AF.Ln)
                    if ec == 0:
                        nc.vector.memset(elg[:, 0:1], LOG_LO)
                    nc.sync.dma_start(eout[:, b, sl], elg[:])

            # ---- finalize: DVE add on logit half, ACT Ln on exp half -
            for t in range(NTL):
                for fc, sl in enumerate(FSL_LG):
                    st = stage_pool.tile([128, FC], f16, tag="st",
                                         name=f"st{t}_{fc}", bufs=2)
                    nc.vector.tensor_scalar_add(st[:], lg[t][:, sl], negc[t][:])
                    nc.sync.dma_start(vout[t, :, sl], st[:])
                for fc, sl in enumerate(FSL_ES):
                    w_ = sl.stop - sl.start
                    st = stage_pool.tile([128, FC], f16, tag="su",
                                         name=f"su{t}_{fc}", bufs=2)
                    nc.scalar.activation(st[:, :w_], es[t][:, sl], AF.Ln,
                                         scale=scl[t][:])
                    nc.sync.dma_start(vout[t, :, LGW + sl.start:LGW + sl.stop],
                                      st[:, :w_])

    nc.compile()
    return nc


def _get_program(has_bout: bool, neg_bcopy: float):
    key = (has_bout, neg_bcopy)
    if key not in _prog_cache:
        _prog_cache[key] = _build_program(has_bout, neg_bcopy)
    return _prog_cache[key]


def _marshal(hidden, attn, copy_to_ext, W_out, b_out, w_copy, b_copy):
    import ml_dtypes
    f8 = ml_dtypes.float8_e4m3

    h2 = np.asarray(hidden, np.float32).reshape(NROWS, HID)
    a2 = np.asarray(attn, np.float32)
    attnT_full = np.ascontiguousarray(
        a2.transpose(1, 2, 0)).astype(np.float16)              # [32, 200, 64]
    idx_full = np.ascontiguousarray(
        np.asarray(copy_to_ext).astype(np.int32).T)            # [32, 200]
    W8 = np.asarray(W_out, np.float32).astype(f8)              # [32000, 1024]
    wc16 = np.asarray(w_copy, np.float32).astype(
        np.float16).reshape(HID)
    bo = np.asarray(b_out, np.float32)
    neg_bcopy = -float(np.asarray(b_copy, np.float32).reshape(-1)[0])
    has_bout = bool(np.any(bo))

    # shared W^T chunks: WTh[vc, p, kb, j] = W.T[kb*128+p, vc*512+j]
    Wt = W8.T                                                  # [1024, 32000]
    full = np.zeros((HID, NVC, VPAD), f8)
    full[:, :NVC - 1, :] = Wt[:, :62 * VPAD].reshape(HID, 62, VPAD)
    full[:, NVC - 1, :VLAST] = Wt[:, 62 * VPAD:]
    WTh = np.ascontiguousarray(
        full.reshape(KB, 128, NVC, VPAD).transpose(2, 1, 0, 3))
    wcT = np.ascontiguousarray(wc16.reshape(KB, 128).T)        # [128, KB]
    if has_bout:
        bbh = np.zeros((1, NVC, VPAD), np.float16)
        bbh[0, :NVC - 1, :] = bo[:62 * VPAD].reshape(62, VPAD)
        bbh[0, NVC - 1, :VLAST] = bo[62 * VPAD:]

    in_maps = []
    for c in range(NCORES):
        # hTh[t, p, kb, r] = h2[c*256 + t*128 + r, kb*128 + p]
        hc = h2[c * RSH:(c + 1) * RSH]
        hcT = np.ascontiguousarray(
            hc.reshape(NTL, 128, KB, 128).transpose(0, 3, 2, 1))
        # hxT[b, p, kb, t] = h2[t*BSZ + (c*BSH+b), kb*128+p]
        hx = np.stack([np.ascontiguousarray(
            h2[(c * BSH + b)::BSZ, :].astype(np.float16)
            .reshape(TLEN, KB, 128).transpose(2, 1, 0)) for b in range(BSH)])
        bsl = slice(c * BSH, (c + 1) * BSH)
        m = {
            "WTh": WTh,
            "hT8": hcT.astype(f8),
            "hTh": hcT.astype(np.float16),
            "wcT": wcT,
            "attnT": np.ascontiguousarray(attnT_full[bsl]),
            "idxc": np.ascontiguousarray(idx_full[bsl]),
            "hxT": hx,
        }
        if has_bout:
            m["bbh"] = bbh
        in_maps.append(m)
    return in_maps, has_bout, neg_bcopy


def _assemble(results):
    out = np.empty((NROWS, V_TGT + V_EXT), np.float32)
    out3 = out.reshape(TLEN, BSZ, V_TGT + V_EXT)
    for c in range(NCORES):
        out[c * RSH:(c + 1) * RSH, :V_TGT] = \
            results[c]["vout"].reshape(RSH, V_TGT)
        out3[:, c * BSH:(c + 1) * BSH, V_TGT:] = results[c]["eout"]
    return out3


LAST_RES = None


def kernel(hidden, attn, copy_to_ext, W_out, b_out, w_copy, b_copy):
    global LAST_RES
    from concourse.bass_utils import run_bass_kernel_spmd

    in_maps, has_bout, neg_bcopy = _marshal(
        hidden, attn, copy_to_ext, W_out, b_out, w_copy, b_copy)
    nc = _get_program(has_bout, neg_bcopy)
    res = run_bass_kernel_spmd(nc, in_maps, core_ids=list(range(NCORES)))
    LAST_RES = res
    return _assemble(res.results)


# revision 19
# speedup vs baseline: 1.4571x; 1.0803x over previous
"""CopyGenerator kernel for 8 Trainium2 NeuronCores.

Sharding (collective-free):
  - Data-parallel over rows for the vocab projection: each core owns
    256 of the 2048 (tlen*bsz) rows and computes the FULL 32k-vocab
    projection + log_softmax locally.  The full W_out streams through
    SBUF in fp8-e4m3 (33 MB, overlapped with the matmuls), so no
    cross-core AllReduce (and no NEFF startup barrier) is needed --
    each core's on-silicon span is its own compute only.
  - Data-parallel over batch for the ext-vocab scatter: 4 of the 32
    batches per core, computed as a onehot matmul (iota + is_equal).

The projection runs fp8 x fp8 DoubleRow on the PE (fp32 PSUM, 2x
rate) in quad-chunk groups: each [128, 2048] 4-bank PSUM tile takes 32
back-to-back matmuls and is drained by ONE wide DVE cast (fp16 logits)
plus ONE wide ACT exp (softmax row-sums), keeping both consumers well
under the PE's production rate.  The copy-gate z uses a separate fp16
matmul for precision.  Host-side work is layout marshalling only.
"""
import sys
sys.path.insert(0, "/opt/trn_rl_repo")
import numpy as np

TLEN, BSZ, HID = 64, 32, 1024
SLEN, V_TGT, V_EXT = 200, 32000, 2000
NCORES = 8
NROWS = TLEN * BSZ             # 2048
RSH = NROWS // NCORES          # 256 rows per core
NTL = RSH // 128               # 2 row tiles per core
BSH = BSZ // NCORES            # 4 batches per core (ext scatter)
KB = HID // 128                # 8 contraction chunks (4 DoubleRow pairs)
VPAD = 512                     # vocab chunk width (psum bank)
GW = 4 * VPAD                  # 2048-col quad group
NG = 16                        # 16 groups (last holds 3 valid chunks)
GLAST = V_TGT - 15 * GW        # 1280 valid cols in the last group
FC = 2048                      # finalize chunk width
FSLS = [slice(i * FC, min(V_TGT, (i + 1) * FC))
        for i in range(-(-V_TGT // FC))]   # 15 full + 1 of 1280
SA, SB_ = 128, SLEN - 128      # source-len split (128 + 72)
EC = 500                       # ext chunk
NEC = V_EXT // EC              # 4
LOG_LO = float(np.log(0.001))

_prog_cache = {}


def _build_program(has_bout: bool, neg_bcopy: float):
    import concourse.bacc as bacc
    import concourse.tile as tile
    import concourse.mybir as mybir

    f32, f16, i32 = mybir.dt.float32, mybir.dt.float16, mybir.dt.int32
    f8 = mybir.dt.float8e4
    AF = mybir.ActivationFunctionType
    OP = mybir.AluOpType
    PM = mybir.MatmulPerfMode

    nc = bacc.Bacc("TRN2", target_bir_lowering=False, debug=False,
                   num_devices=NCORES)

    # tile-contiguous host layouts
    WTh = nc.dram_tensor("WTh", [NG, 128, KB, GW], f8, kind="ExternalInput")
    hT8 = nc.dram_tensor("hT8", [NTL, 128, KB, 128], f8, kind="ExternalInput")
    hTh = nc.dram_tensor("hTh", [NTL, 128, KB, 128], f16, kind="ExternalInput")
    wcT = nc.dram_tensor("wcT", [128, KB], f16, kind="ExternalInput")
    attnT = nc.dram_tensor("attnT", [BSH, SLEN, TLEN], f16, kind="ExternalInput")
    idxc = nc.dram_tensor("idxc", [BSH, SLEN], i32, kind="ExternalInput")
    hxT = nc.dram_tensor("hxT", [BSH, 128, KB, TLEN], f16, kind="ExternalInput")
    if has_bout:
        bbh = nc.dram_tensor("bbh", [1, NG, GW], f16, kind="ExternalInput")
    vout = nc.dram_tensor("vout", [NTL, 128, V_TGT], f16, kind="ExternalOutput")
    eout = nc.dram_tensor("eout", [TLEN, BSH, V_EXT], f32, kind="ExternalOutput")

    with tile.TileContext(nc) as tc:
        with (
            tc.tile_pool(name="lg", bufs=NTL) as lg_pool,
            tc.tile_pool(name="wt", bufs=2) as wt_pool,
            tc.tile_pool(name="ht", bufs=2 * NTL) as ht_pool,
            tc.tile_pool(name="const", bufs=1) as const_pool,
            tc.tile_pool(name="esc", bufs=2) as esc_pool,
            tc.tile_pool(name="sep", bufs=NTL) as sep_pool,
            tc.tile_pool(name="stage", bufs=4) as stage_pool,
            tc.tile_pool(name="small", bufs=8) as small_pool,
            tc.tile_pool(name="ext", bufs=2) as ext_pool,
            tc.tile_pool(name="ps", bufs=2, space="PSUM") as ps_pool,
        ):
            # ---- prologue loads --------------------------------------
            ht8, ht = {}, {}
            for t in range(NTL):
                ht8[t] = ht_pool.tile([128, KB, 128], f8, tag="ht8",
                                      name=f"ht8_{t}")
                nc.gpsimd.dma_start(ht8[t][:], hT8[t])
                ht[t] = ht_pool.tile([128, KB, 128], f16, tag="ht",
                                     name=f"ht{t}")
                nc.gpsimd.dma_start(ht[t][:], hTh[t])
            wc_sb = const_pool.tile([128, KB], f16)
            nc.gpsimd.dma_start(wc_sb[:], wcT[:])
            if has_bout:
                bb_sb = const_pool.tile([1, NG, GW], f16)
                nc.gpsimd.dma_start(bb_sb[:], bbh[0])
                ones1 = const_pool.tile([1, 128], f16)
                nc.vector.memset(ones1[:], 1.0)

            lg = {t: lg_pool.tile([128, V_TGT], f16, tag="lg", name=f"lg{t}")
                  for t in range(NTL)}
            sep = {t: sep_pool.tile([128, NG], f32, tag="sep",
                                    name=f"sep{t}") for t in range(NTL)}

            # copy-gate z via fp16 matmul (separate from fp8 stream)
            zc = {}
            for t in range(NTL):
                zp = ps_pool.tile([128, GW], f32, tag="pm", name=f"zp{t}")
                for kb in range(KB):
                    nc.tensor.matmul(zp[:, :1], ht[t][:, kb, :],
                                     wc_sb[:, kb:kb + 1],
                                     start=(kb == 0), stop=(kb == KB - 1))
                e_t = small_pool.tile([128, 1], f32, tag="e", name=f"e{t}")
                nc.scalar.activation(e_t[:], zp[:, :1], AF.Exp,
                                     scale=-1.0, bias=neg_bcopy)
                zc[t] = e_t

            # ext gate (1 - sigmoid) per batch, early so its Exp shares
            # the main Exp table epoch
            sgxs = {}
            for b in range(BSH):
                hx_sb = ext_pool.tile([128, KB, TLEN], f16, tag="hx", bufs=1)
                nc.gpsimd.dma_start(hx_sb[:], hxT[b])
                zx = ps_pool.tile([128, GW], f32, tag="pm", name=f"zx{b}")
                for kb in range(KB):
                    nc.tensor.matmul(zx[:TLEN, :1], hx_sb[:, kb, :],
                                     wc_sb[:, kb:kb + 1],
                                     start=(kb == 0), stop=(kb == KB - 1))
                ex = small_pool.tile([TLEN, 1], f32, tag="ex", name=f"ex{b}")
                nc.scalar.activation(ex[:], zx[:TLEN, :1], AF.Exp,
                                     scale=-1.0, bias=neg_bcopy)
                spx = small_pool.tile([TLEN, 1], f32, tag="spx", name=f"spx{b}")
                nc.vector.tensor_scalar_add(spx[:], ex[:], 1.0)
                ivx = small_pool.tile([TLEN, 1], f32, tag="ivx", name=f"ivx{b}")
                nc.vector.reciprocal(ivx[:], spx[:])
                sgx = small_pool.tile([TLEN, 1], f32, tag="sgx", name=f"sgx{b}")
                nc.vector.tensor_mul(sgx[:], ex[:], ivx[:])   # 1 - sigmoid
                sgxs[b] = sgx

            # ---- main loop: stream W quad groups, both row tiles -----
            for g in range(NG):
                w = wt_pool.tile([128, KB, GW], f8, tag="wt", name=f"wt{g}")
                nc.gpsimd.dma_start(w[:], WTh[g])
                wtot = GLAST if g == NG - 1 else GW
                nsub = -(-wtot // VPAD)
                for t in range(NTL):
                    pm = ps_pool.tile([128, GW], f32, tag="pm",
                                      name=f"pm{g}_{t}")
                    for i in range(nsub):
                        po = pm[:, i * VPAD:(i + 1) * VPAD]
                        for kp in range(KB // 2):
                            nc.tensor.matmul(
                                po, ht8[t][:, 2 * kp:2 * kp + 2, :],
                                w[:, 2 * kp:2 * kp + 2,
                                  i * VPAD:(i + 1) * VPAD],
                                start=(kp == 0),
                                stop=(kp == KB // 2 - 1 and not has_bout),
                                perf_mode=PM.DoubleRow)
                        if has_bout:
                            nc.tensor.matmul(
                                po, ones1[:],
                                bb_sb[:, g, i * VPAD:(i + 1) * VPAD],
                                start=False, stop=True)
                    # psum -> fp16 logits (DVE); exp row-sums (ACT)
                    nc.vector.tensor_copy(lg[t][:, g * GW:g * GW + wtot],
                                          pm[:, :wtot])
                    esc = esc_pool.tile([128, GW], f16, tag="esc",
                                        name=f"esc{g}_{t}", bufs=1)
                    nc.scalar.activation(esc[:, :wtot], pm[:, :wtot],
                                         AF.Exp,
                                         accum_out=sep[t][:, g:g + 1])

            # ---- gate + normalizer per row tile ----------------------
            negc = {}
            for t in range(NTL):
                sp = small_pool.tile([128, 1], f32, tag="sp", name=f"sp{t}")
                nc.vector.tensor_scalar_add(sp[:], zc[t][:], 1.0)
                sig = small_pool.tile([128, 1], f32, tag="sig", name=f"sig{t}")
                nc.vector.reciprocal(sig[:], sp[:])
                cl = small_pool.tile([128, 1], f32, tag="cl", name=f"cl{t}")
                nc.vector.tensor_scalar(cl[:], sig[:], 0.001, 0.999,
                                        op0=OP.max, op1=OP.min)
                lcs = small_pool.tile([128, 1], f32, tag="lcs", name=f"lcs{t}")
                nc.scalar.activation(lcs[:], cl[:], AF.Ln)
                ssum = small_pool.tile([128, 1], f32, tag="ssum",
                                       name=f"ssum{t}")
                nc.vector.tensor_reduce(ssum[:], sep[t][:],
                                        axis=mybir.AxisListType.X, op=OP.add)
                lns = small_pool.tile([128, 1], f32, tag="lns", name=f"lns{t}")
                nc.scalar.activation(lns[:], ssum[:], AF.Ln)
                ng_ = small_pool.tile([128, 1], f32, tag="negc",
                                      name=f"negc{t}")
                nc.vector.tensor_sub(ng_[:], lcs[:], lns[:])
                negc[t] = ng_

            # ---- ext-vocab scatter (batch-sharded) --------------------
            iota_sb = const_pool.tile([128, V_EXT], f32)
            nc.gpsimd.iota(iota_sb[:], pattern=[[1, V_EXT]], base=0,
                           channel_multiplier=0,
                           allow_small_or_imprecise_dtypes=True)
            for b in range(BSH):
                sgx = sgxs[b]
                idx_i = ext_pool.tile([128, 2], i32, tag="idxi")
                nc.sync.dma_start(idx_i[:SA, 0:1],
                                  idxc[b:b + 1, 0:SA].rearrange("o s -> s o"))
                nc.sync.dma_start(idx_i[:SB_, 1:2],
                                  idxc[b:b + 1, SA:SLEN].rearrange("o s -> s o"))
                idx_sb = ext_pool.tile([128, 2], f32, tag="idx")
                nc.vector.tensor_copy(idx_sb[:SA, 0:1], idx_i[:SA, 0:1])
                nc.vector.tensor_copy(idx_sb[:SB_, 1:2], idx_i[:SB_, 1:2])
                oh_a = ext_pool.tile([128, V_EXT], f16, tag="oha", bufs=1)
                oh_b = ext_pool.tile([128, V_EXT], f16, tag="ohb", bufs=1)
                nc.vector.tensor_scalar(oh_a[:], iota_sb[:], idx_sb[:, 0:1],
                                        None, op0=OP.is_equal)
                nc.vector.tensor_scalar(oh_b[:SB_], iota_sb[:SB_],
                                        idx_sb[:SB_, 1:2], None,
                                        op0=OP.is_equal)

                at_a = ext_pool.tile([128, TLEN], f16, tag="ata")
                at_b = ext_pool.tile([128, TLEN], f16, tag="atb")
                nc.gpsimd.dma_start(at_a[:], attnT[b, 0:SA, :])
                nc.gpsimd.dma_start(at_b[:SB_], attnT[b, SA:SLEN, :])

                pe_ = ps_pool.tile([128, GW], f32, tag="pm", name=f"pe{b}")
                for ec in range(NEC):
                    sl = slice(ec * EC, (ec + 1) * EC)
                    po = pe_[:TLEN, ec * VPAD:ec * VPAD + EC]
                    nc.tensor.matmul(po, at_a[:], oh_a[:, sl],
                                     start=True, stop=False)
                    nc.tensor.matmul(po, at_b[:SB_], oh_b[:SB_, sl],
                                     start=False, stop=True)
                for ec in range(NEC):
                    po = pe_[:TLEN, ec * VPAD:ec * VPAD + EC]
                    sl = slice(ec * EC, (ec + 1) * EC)
                    est = stage_pool.tile([TLEN, EC], f32, tag="est",
                                          name=f"est{b}_{ec}", bufs=1)
                    nc.vector.tensor_scalar(est[:], po, sgx[:],
                                            0.001, op0=OP.mult, op1=OP.max)
                    nc.vector.tensor_scalar_min(est[:], est[:], 0.999)
                    elg = stage_pool.tile([TLEN, EC], f32, tag="elg",
                                          name=f"elg{b}_{ec}", bufs=2)
                    nc.scalar.activation(elg[:], est[:], AF.Ln)
                    if ec == 0:
                        nc.vector.memset(elg[:, 0:1], LOG_LO)
                    nc.sync.dma_start(eout[:, b, sl], elg[:])

            # ---- finalize: out = logits + negc, write vout (DVE) -----
            for t in range(NTL):
                for fc, sl in enumerate(FSLS):
                    w_ = sl.stop - sl.start
                    st = stage_pool.tile([128, FC], f16, tag="st",
                                         name=f"st{t}_{fc}", bufs=2)
                    nc.vector.tensor_scalar_add(st[:, :w_], lg[t][:, sl],
                                                negc[t][:])
                    nc.sync.dma_start(vout[t, :, sl], st[:, :w_])

    nc.compile()
    return nc


def _get_program(has_bout: bool, neg_bcopy: float):
    key = (has_bout, neg_bcopy)
    if key not in _prog_cache:
        _prog_cache[key] = _build_program(has_bout, neg_bcopy)
    return _prog_cache[key]


def _marshal(hidden, attn, copy_to_ext, W_out, b_out, w_copy, b_copy):
    import ml_dtypes
    f8 = ml_dtypes.float8_e4m3

    h2 = np.asarray(hidden, np.float32).reshape(NROWS, HID)
    a2 = np.asarray(attn, np.float32)
    attnT_full = np.ascontiguousarray(
        a2.transpose(1, 2, 0)).astype(np.float16)              # [32, 200, 64]
    idx_full = np.ascontiguousarray(
        np.asarray(copy_to_ext).astype(np.int32).T)            # [32, 200]
    W8 = np.asarray(W_out, np.float32).astype(f8)              # [32000, 1024]
    wc16 = np.asarray(w_copy, np.float32).astype(
        np.float16).reshape(HID)
    bo = np.asarray(b_out, np.float32)
    neg_bcopy = -float(np.asarray(b_copy, np.float32).reshape(-1)[0])
    has_bout = bool(np.any(bo))

    # shared W^T groups: WTh[g, p, kb, j] = W.T[kb*128+p, g*2048+j]
    Wt = W8.T                                                  # [1024, 32000]
    full = np.zeros((HID, NG, GW), f8)
    fl = full.reshape(HID, NG * GW)
    fl[:, :V_TGT] = Wt
    WTh = np.ascontiguousarray(
        full.reshape(KB, 128, NG, GW).transpose(2, 1, 0, 3))
    wcT = np.ascontiguousarray(wc16.reshape(KB, 128).T)        # [128, KB]
    if has_bout:
        bbh = np.zeros((1, NG, GW), np.float16)
        bbh.reshape(1, NG * GW)[0, :V_TGT] = bo
    in_maps = []
    for c in range(NCORES):
        # hTh[t, p, kb, r] = h2[c*256 + t*128 + r, kb*128 + p]
        hc = h2[c * RSH:(c + 1) * RSH]
        hcT = np.ascontiguousarray(
            hc.reshape(NTL, 128, KB, 128).transpose(0, 3, 2, 1))
        # hxT[b, p, kb, t] = h2[t*BSZ + (c*BSH+b), kb*128+p]
        hx = np.stack([np.ascontiguousarray(
            h2[(c * BSH + b)::BSZ, :].astype(np.float16)
            .reshape(TLEN, KB, 128).transpose(2, 1, 0)) for b in range(BSH)])
        bsl = slice(c * BSH, (c + 1) * BSH)
        m = {
            "WTh": WTh,
            "hT8": hcT.astype(f8),
            "hTh": hcT.astype(np.float16),
            "wcT": wcT,
            "attnT": np.ascontiguousarray(attnT_full[bsl]),
            "idxc": np.ascontiguousarray(idx_full[bsl]),
            "hxT": hx,
        }
        if has_bout:
            m["bbh"] = bbh
        in_maps.append(m)
    return in_maps, has_bout, neg_bcopy


def _assemble(results):
    out = np.empty((NROWS, V_TGT + V_EXT), np.float32)
    out3 = out.reshape(TLEN, BSZ, V_TGT + V_EXT)
    for c in range(NCORES):
        out[c * RSH:(c + 1) * RSH, :V_TGT] = \
            results[c]["vout"].reshape(RSH, V_TGT)
        out3[:, c * BSH:(c + 1) * BSH, V_TGT:] = results[c]["eout"]
    return out3


LAST_RES = None


def kernel(hidden, attn, copy_to_ext, W_out, b_out, w_copy, b_copy):
    global LAST_RES
    from concourse.bass_utils import run_bass_kernel_spmd

    in_maps, has_bout, neg_bcopy = _marshal(
        hidden, attn, copy_to_ext, W_out, b_out, w_copy, b_copy)
    nc = _get_program(has_bout, neg_bcopy)
    res = run_bass_kernel_spmd(nc, in_maps, core_ids=list(range(NCORES)))
    LAST_RES = res
    return _assemble(res.results)
